# revision 17
# baseline (speedup 1.0000x reference)
"""Binary position embedding kernel for Trainium2, 8-core data-parallel.

out[t, :] = sum_b bit_b(x[t]) * weight[b, :]  ==  bits(x) @ weight

v5: fp16 end-to-end on the device (the 2e-2 rel-err budget dwarfs fp16's
~3e-4), which halves the output stream to 8 MiB/core — the kernel is
DMA-write-bound, so this is the dominant win over the f32 baseline.

Sharding: x flat [32768] -> 8 shards of 4096 tokens; weight replicated.

Per-core plan (4096 tokens -> [4096, 1024] fp16 = 8 MiB output):
  - Two 32-row PE groups (tile rows 0 and 32). Supertile = 256 tokens =
    2 groups x 128; group g computes tokens {s*256 + 2p + g} (host
    permutes x). PSUM: ONE [128, 2048] f32 tag (4 banks) x bufs=2 =
    all 8 banks; both groups' 4 matmuls fill one buffer and supertile
    s+2's matmuls only wait on supertile s's copy — the mm->copy->mm
    loop never gates the stream, and one wide [128, 2048] copy per
    supertile amortizes the engines' fixed PSUM-access cost.
  - Inputs spread over all 3 DGE queues, hoisted before the Tile entry
    barrier to overlap the fixed ~6.5 us NRT preamble, and split so the
    pieces that unblock the first supertile land first: ACT's HWDGE
    queue carries wm [45, 1028] i16 as shift/mask columns (360 B) then
    weight columns; SP's queue carries x cols [0:512] then [512:4096]
    for group 0; gpsimd SWDGE the same two pieces for group 1. Dead
    rows between the groups are never read downstream, so their
    uninitialized x/mask partitions are harmless.
  - bits: ONE fused bitwise DVE tensor_scalar per column chunk,
    (x << shift[p]) & fp16_bit_mask[p], which lands each bit at an fp16
    power-of-two bit position; the host prescales weight rows by the
    matching power of two, so the int16 result bitcast to fp16 IS the
    bit matrix (no arith cast op needed). Chunks are emitted interleaved
    with the copy stream so early copies aren't stuck behind bits for
    late tokens.
  - PSUM -> SBUF copies cast f32 -> fp16, one [128, 2048] copy per
    supertile split ACT / DVE (DVE also runs bits); the first and last
    supertiles use per-group [128, 1024] sub-copies on both engines for
    latency at the stream's ends.
  - Output: token interleave makes partition p's 4 KiB contiguous in
    DRAM, so each supertile is ONE [128, 2048] fp16 DMA (the f32
    baseline's per-row 4 KiB descriptors made the E79 descriptor-
    dispatch engine's per-packet overhead an ~8 us serial tail; 4 KiB
    descriptors with half the packets stay clear of it). The first and
    last supertiles issue per-group DMAs for latency at the stream's
    ends. All output DMAs issue on SP after its single x issue.
"""

import numpy as np

import concourse.bass as bass
import concourse.mybir as mybir
from concourse.bass_utils import run_bass_kernel_spmd
from concourse.tile import TileContext
from concourse.vector_clock import ScopedClock


class _LeanTailTileContext(TileContext):
    """Standard tail emits drain -> barrier -> sem clears -> barrier. The
    final barrier only syncs engine-stream ends after the gpsimd-only sem
    clears; dropping it shaves the second EVSEM butterfly off the critical
    path. Re-execution stays safe: clears still run after the full barrier,
    and the next run's entry barrier resynchronizes engines."""

    def _drain_and_barrier(self, tick_clock, wait_clock):
        nc = self.nc
        drain_inst = nc.sync.drain()
        wait_clock.add_sem_waits(
            drain_inst.ins, ScopedClock({None: tick_clock.global_clock})
        )
        nc.all_engine_barrier()
        popped = nc._tile_sem_poison_stack.pop()
        assert popped is self._sem_poison
        nc.clear_and_free_semaphores(list(self.sems.allocated().values()))


N_CORES = 8
B, S, D = 4, 8192, 1024
NB = 13                    # bits per position
NG = 2                     # PE row groups / token interleave factor
TOK = (B * S) // N_CORES   # 4096 tokens per core
TILE = 128
ST = NG * TILE             # 256 tokens per supertile
NST = TOK // ST            # 16 supertiles
NPART = 32 + NB            # wm rows transferred (45); dead rows 13..31 unread

W_COLS = D                 # wm cols [0:1024] = prescaled fp16 weight bitcast
SHIFT_COL = W_COLS         # wm col 1024 = left-shift amount
MASK_COL = W_COLS + 1      # wm col 1025 = fp16 power-of-two bit pattern
WM_COLS = W_COLS + 4       # pad to 4-col alignment

# Row r carries bit r, left-shifted to fp16 bit position PB so the fused
# bitwise (x << shift) & mask yields the fp16 value of pattern 1<<PB
# directly; the weight row is host-prescaled by 1/that (exact powers of
# two: 2^-14 / 2^-13 / 2^-11 for PB 10 / 11 / 12).
PB = [10] * 11 + [11, 12]

TRACE = False
LAST_RESULTS = None

_wsplit_counter = [0]


def _split_multi_waits(nc):
    """This env's walrus allows only one sync-wait per instruction. Hoist
    extra semaphore waits onto single-wait NoOps inserted just before the
    instruction on the same engine stream (same per-engine program order,
    identical blocking semantics)."""
    import bass_rust

    n_split = 0
    for f in nc.m.functions:
        for bb in f.blocks:
            insts = bb.instructions
            i = 0
            while i < len(insts):
                ins = insts[i]
                si = ins.sync_info
                if si is not None:
                    waits = list(si.on_wait)
                    sem_waits = [w for w in waits if w.sync_type == "semaphore"]
                    other = [w for w in waits if w.sync_type != "semaphore"]
                    keep = 1 if not other else 0
                    if len(waits) > 1 and len(sem_waits) > keep:
                        hoist = sem_waits[: len(sem_waits) - keep]
                        kept = sem_waits[len(sem_waits) - keep:]
                        si.on_wait = other + kept
                        for w in hoist:
                            noop = mybir.InstNoOp(
                                name=f"wsplit-{_wsplit_counter[0]}", ins=[], outs=[]
                            )
                            _wsplit_counter[0] += 1
                            noop.engine = ins.engine
                            noop.sync_info = bass_rust.SyncInfo(
                                on_wait=[w], on_update=[]
                            )
                            insts.insert(i, noop)
                            i += 1
                            n_split += 1
                i += 1
    return n_split


def _drop_entry_barrier(nc):
    """Remove the Tile entry barrier (per-engine Drain + EVSEM butterfly) from
    the preamble block. The preamble's RegisterMoves are same-engine/program-
    order with the body, its memset'd const tiles have no readers, and every
    real cross-engine dependency in the body is semaphore-gated, so the
    barrier only adds latency (~0.2-0.5 us on the critical engine)."""
    main = nc.m.functions[0].blocks[0]
    insts = main.instructions
    i, n = 0, 0
    while i < len(insts):
        ins = insts[i]
        if ins.opcode == "Drain" or ins.name.startswith("barrier_"):
            insts.pop(i)
            n += 1
        else:
            i += 1
    return n


def _hoist_to_preamble(nc, names):
    """Move the named (wait-free) instructions from the body block to the
    preamble block, before the Tile entry barrier, so their DMA transfers
    overlap the fixed kernel-start overhead."""
    main_bb = nc.m.functions[0].blocks[0]
    moved = []
    for f in nc.m.functions:
        for bb in f.blocks:
            if bb is main_bb:
                continue
            insts = bb.instructions
            i = 0
            while i < len(insts):
                if insts[i].name in names:
                    moved.append(insts.pop(i))
                else:
                    i += 1
    pos = 0
    mi = main_bb.instructions
    while pos < len(mi) and mi[pos].opcode in ("Call", "RegisterMove"):
        pos += 1
    for j, ins in enumerate(moved):
        mi.insert(pos + j, ins)
    return len(moved)


def _build():
    f16 = mybir.dt.float16
    f32 = mybir.dt.float32
    i16 = mybir.dt.int16
    op = mybir.AluOpType

    nc = bass.Bass()
    wm = nc.declare_dram_parameter("wm", [NPART, WM_COLS], i16, isOutput=False)
    xsrc = nc.declare_dram_parameter("xsrc", [NB, TOK], i16, isOutput=False)
    out = nc.declare_dram_parameter("out", [TOK, D], f16, isOutput=True)

    hoist_names = []
    with _LeanTailTileContext(nc) as tc:
        with (
            tc.tile_pool(name="const", bufs=1) as cpool,
            tc.tile_pool(name="outp", bufs=8) as opool,
            tc.tile_pool(name="psum", bufs=1, space="PSUM") as ppool,
        ):
            sb = cpool.tile([64, WM_COLS], i16)
            xb = cpool.tile([64, TOK], i16)
            bt = cpool.tile([64, TOK], i16)

            w = sb[:, 0:W_COLS].bitcast(f16)
            shf = sb[:, SHIFT_COL : SHIFT_COL + 1]
            mks = sb[:, MASK_COL : MASK_COL + 1]
            btf = bt.bitcast(f16)

            # input DMAs (hoisted to the preamble by name below), spread
            # over the three DGE queues, first-needed pieces first
            XC = 512  # x columns needed by supertiles 0-1
            dmas = [
                nc.scalar.dma_start(
                    sb[0:NPART, W_COLS:WM_COLS], wm[:, W_COLS:WM_COLS]
                ),
                nc.scalar.dma_start(sb[0:NPART, 0:W_COLS], wm[:, 0:W_COLS]),
                nc.sync.dma_start(xb[0:NB, 0:XC], xsrc[:, 0:XC]),
                nc.sync.dma_start(xb[0:NB, XC:], xsrc[:, XC:]),
                nc.gpsimd.dma_start(xb[32 : 32 + NB, 0:XC], xsrc[:, 0:XC]),
                nc.gpsimd.dma_start(xb[32 : 32 + NB, XC:], xsrc[:, XC:]),
            ]
            hoist_names = [d.ins.name for d in dmas]

            # bits: ONE fused bitwise (x << shift) & mask per chunk on DVE
            # (2x 16-bit mode) writes the fp16 bit pattern directly. Dead
            # rows have mask 0, so their uninitialized x reads land as
            # exact zeros.
            def bits(lo, hi, plo, phi):
                nc.vector.tensor_scalar(
                    bt[plo:phi, lo:hi], xb[plo:phi, lo:hi],
                    shf[plo:phi], mks[plo:phi],
                    op.logical_shift_left, op.bitwise_and,
                )

            DVE_BIG = {2, 4, 6, 9, 12, 14}

            def supertile(s):
                ob = opool.tile([TILE, NG * D], f16)
                split = s in (0, NST - 1)
                pt = ppool.tile([TILE, NG * 1024], f32, tag="A", bufs=2)
                for g in range(NG):
                    c0 = (s * NG + g) * TILE
                    for h in range(2):
                        nc.tensor.matmul(
                            pt[:, g * 1024 + 512 * h : g * 1024 + 512 * (h + 1)],
                            btf[32 * g : 32 * g + NB, c0 : c0 + TILE],
                            w[32 * g : 32 * g + NB, 512 * h : 512 * (h + 1)],
                            start=True,
                            stop=True,
                            tile_position=(32 * g, 0),
                        )
                    if split:
                        # per-group sub-copy + per-group DMA: lowest latency
                        # at the stream's ends
                        dst = ob[:, g * D : (g + 1) * D]
                        src_ = pt[:, g * 1024 : (g + 1) * 1024]
                        if g == 0:
                            nc.scalar.copy(dst, src_)
                        else:
                            nc.vector.tensor_copy(dst, src_)
                        dv = out[s * ST : (s + 1) * ST, :].rearrange(
                            "(p g) d -> p g d", g=NG
                        )[:, g : g + 1, :]
                        nc.sync.dma_start(dv, dst)
                if not split:
                    if s in DVE_BIG:
                        nc.vector.tensor_copy(ob[:], pt[:])
                    else:
                        nc.scalar.copy(ob[:], pt[:])
                    dram_view = out[s * ST : (s + 1) * ST, :].rearrange(
                        "(p g) d -> p (g d)", g=NG
                    )
                    nc.sync.dma_start(dram_view, ob[:])

            # interleave bits chunks with the supertile stream so DVE's
            # copy ladder isn't stuck behind bits for late tokens
            bits(0, 512, 0, 32)
            bits(0, 512, 32, 64)
            supertile(0)
            supertile(1)
            bits(512, 1024, 0, 64)
            supertile(2)
            supertile(3)
            bits(1024, 2048, 0, 64)
            supertile(4)
            supertile(5)
            bits(2048, 4096, 0, 64)
            for s in range(6, NST):
                supertile(s)

    _hoist_to_preamble(nc, set(hoist_names))
    _drop_entry_barrier(nc)
    _split_multi_waits(nc)
    return nc


_nc_cache = None


def _make_wm(weight):
    """[NPART, WM_COLS] int16: prescaled fp16 weight rows (bitcast) plus
    per-row left-shift amounts and fp16 single-bit masks, replicated into
    both 32-row groups. Row r's bit lands at fp16 bit position PB[r]
    (pattern 1 << PB[r]); the weight row is prescaled by 1/value(pattern)
    — exact powers of two, no precision loss."""
    wmk = np.zeros((NPART, WM_COLS), np.int16)
    pb = np.array(PB)
    pat_val = np.array(
        [np.frombuffer(np.int16(1 << p).tobytes(), np.float16)[0] for p in pb],
        dtype=np.float32,
    )
    w16 = (np.asarray(weight, dtype=np.float32) / pat_val[:, None]).astype(np.float16)
    shifts = (pb - np.arange(NB)).astype(np.int16)
    masks = (1 << pb).astype(np.int16)
    for g in range(NG):
        wmk[32 * g : 32 * g + NB, 0:W_COLS] = w16.view(np.int16)
        wmk[32 * g : 32 * g + NB, SHIFT_COL] = shifts
        wmk[32 * g : 32 * g + NB, MASK_COL] = masks
    return wmk


def kernel(x, weight):
    global _nc_cache, LAST_RESULTS
    if _nc_cache is None:
        _nc_cache = _build()
    nc = _nc_cache
    wmk = _make_wm(weight)

    # x values are < 8192 so they fit int16 exactly. Within each supertile
    # of 256 tokens, bits column (2s+g)*128 + p must hold token
    # s*256 + 2p + g so each DRAM partition row is 4 KiB contiguous.
    xf = np.asarray(x, dtype=np.int32).reshape(-1).astype(np.int16)
    in_maps = []
    for c in range(N_CORES):
        xs = xf[c * TOK : (c + 1) * TOK]
        xperm = xs.reshape(NST, TILE, NG).swapaxes(1, 2).reshape(-1)
        in_maps.append(
            {
                "wm": wmk,
                "xsrc": np.broadcast_to(xperm, (NB, TOK)).copy(),
            }
        )
    res = run_bass_kernel_spmd(nc, in_maps, list(range(N_CORES)), trace=TRACE)
    LAST_RESULTS = res
    out = np.concatenate([r["out"] for r in res.results], axis=0)
    return out.astype(np.float32).reshape(B, S, D)


# revision 18
# speedup vs baseline: 1.0545x; 1.0545x over previous
"""Binary position embedding kernel for Trainium2, 8-core data-parallel.

out[t, :] = sum_b bit_b(x[t]) * weight[b, :]  ==  bits(x) @ weight

v6: fp16 end-to-end on the device (the 2e-2 rel-err budget dwarfs fp16's
~3e-4), which halves the output stream to 8 MiB/core — the kernel is
DMA-write-bound, so this is the dominant win over the f32 baseline.

Sharding: x flat [32768] -> 8 shards of 4096 tokens; weight replicated.

Per-core plan (4096 tokens -> [4096, 1024] fp16 = 8 MiB output):
  - Two 32-row PE groups (tile rows 0 and 32). Supertile = 256 tokens =
    2 groups x 128; group g computes tokens {s*256 + 2p + g} (host
    permutes x). PSUM: 2 tags x bufs=2 x [128, 1024] f32 (2 banks) =
    all 8 banks, so group g's matmuls for supertile s+1 only wait on
    its copy from supertile s-1 — the mm->copy->mm loop never gates
    the stream.
  - Inputs: just 3 DMAs, one per DGE queue, all hoisted before the Tile
    entry barrier to overlap the fixed ~6.5 us NRT preamble: wm [64,
    1028] i16 (prescaled fp16 weight rows bitcast + per-row shift/mask,
    both groups) on ACT's HWDGE queue; x [13, 4096] i16 to group 0's
    partitions on SP's queue and group 1's via gpsimd SWDGE. Dead rows
    have mask 0, which zeroes whatever garbage their uninitialized x
    partitions hold.
  - bits: ONE fused bitwise DVE tensor_scalar per column chunk,
    (x << shift[p]) & fp16_bit_mask[p], which lands each bit at an fp16
    power-of-two bit position; the host prescales weight rows by the
    matching power of two, so the int16 result bitcast to fp16 IS the
    bit matrix (no arith cast op needed). Chunks are emitted interleaved
    with the copy stream so early copies aren't stuck behind bits for
    late tokens.
  - PSUM -> SBUF copies cast f32 -> fp16, one [128, 1024] copy per
    (s, g), split ACT (g0) / DVE (g1, which also runs bits).
  - Output: token interleave makes partition p's 4 KiB contiguous in
    DRAM, so each supertile is ONE [128, 2048] fp16 DMA (the f32
    baseline's per-row 4 KiB descriptors made the E79 descriptor-
    dispatch engine's per-packet overhead an ~8 us serial tail; 4 KiB
    descriptors with half the packets stay clear of it). The first and
    last supertiles issue per-group DMAs for latency at the stream's
    ends. All output DMAs issue on SP after its single x issue.
"""

import numpy as np

import concourse.bass as bass
import concourse.mybir as mybir
from concourse.bass_utils import run_bass_kernel_spmd
from concourse.tile import TileContext
from concourse.vector_clock import ScopedClock


class _LeanTailTileContext(TileContext):
    """Standard tail emits drain -> barrier -> sem clears -> barrier. The
    final barrier only syncs engine-stream ends after the gpsimd-only sem
    clears; dropping it shaves the second EVSEM butterfly off the critical
    path. Re-execution stays safe: clears still run after the full barrier,
    and the next run's entry barrier resynchronizes engines."""

    def _drain_and_barrier(self, tick_clock, wait_clock):
        nc = self.nc
        drain_inst = nc.sync.drain()
        wait_clock.add_sem_waits(
            drain_inst.ins, ScopedClock({None: tick_clock.global_clock})
        )
        nc.all_engine_barrier()
        popped = nc._tile_sem_poison_stack.pop()
        assert popped is self._sem_poison
        nc.clear_and_free_semaphores(list(self.sems.allocated().values()))


N_CORES = 8
B, S, D = 4, 8192, 1024
NB = 13                    # bits per position
NG = 2                     # PE row groups / token interleave factor
TOK = (B * S) // N_CORES   # 4096 tokens per core
TILE = 128
ST = NG * TILE             # 256 tokens per supertile
NST = TOK // ST            # 16 supertiles
NPART = 32 + NB            # wm rows transferred (45); dead rows 13..31 unread

W_COLS = D                 # wm cols [0:1024] = prescaled fp16 weight bitcast
SHIFT_COL = W_COLS         # wm col 1024 = left-shift amount
MASK_COL = W_COLS + 1      # wm col 1025 = fp16 power-of-two bit pattern
WM_COLS = W_COLS + 4       # pad to 4-col alignment

# Row r carries bit r, left-shifted to fp16 bit position PB so the fused
# bitwise (x << shift) & mask yields the fp16 value of pattern 1<<PB
# directly; the weight row is host-prescaled by 1/that (exact powers of
# two: 2^-14 / 2^-13 / 2^-11 for PB 10 / 11 / 12).
PB = [10] * 11 + [11, 12]

TRACE = False
LAST_RESULTS = None

_wsplit_counter = [0]


def _split_multi_waits(nc):
    """This env's walrus allows only one sync-wait per instruction. Hoist
    extra semaphore waits onto single-wait NoOps inserted just before the
    instruction on the same engine stream (same per-engine program order,
    identical blocking semantics)."""
    import bass_rust

    n_split = 0
    for f in nc.m.functions:
        for bb in f.blocks:
            insts = bb.instructions
            i = 0
            while i < len(insts):
                ins = insts[i]
                si = ins.sync_info
                if si is not None:
                    waits = list(si.on_wait)
                    sem_waits = [w for w in waits if w.sync_type == "semaphore"]
                    other = [w for w in waits if w.sync_type != "semaphore"]
                    keep = 1 if not other else 0
                    if len(waits) > 1 and len(sem_waits) > keep:
                        hoist = sem_waits[: len(sem_waits) - keep]
                        kept = sem_waits[len(sem_waits) - keep:]
                        si.on_wait = other + kept
                        for w in hoist:
                            noop = mybir.InstNoOp(
                                name=f"wsplit-{_wsplit_counter[0]}", ins=[], outs=[]
                            )
                            _wsplit_counter[0] += 1
                            noop.engine = ins.engine
                            noop.sync_info = bass_rust.SyncInfo(
                                on_wait=[w], on_update=[]
                            )
                            insts.insert(i, noop)
                            i += 1
                            n_split += 1
                i += 1
    return n_split


def _drop_entry_barrier(nc):
    """Remove the Tile entry barrier (per-engine Drain + EVSEM butterfly) from
    the preamble block. The preamble's RegisterMoves are same-engine/program-
    order with the body, its memset'd const tiles have no readers, and every
    real cross-engine dependency in the body is semaphore-gated, so the
    barrier only adds latency (~0.2-0.5 us on the critical engine)."""
    main = nc.m.functions[0].blocks[0]
    insts = main.instructions
    i, n = 0, 0
    while i < len(insts):
        ins = insts[i]
        if ins.opcode == "Drain" or ins.name.startswith("barrier_"):
            insts.pop(i)
            n += 1
        else:
            i += 1
    return n


def _hoist_to_preamble(nc, names):
    """Move the named (wait-free) instructions from the body block to the
    preamble block, before the Tile entry barrier, so their DMA transfers
    overlap the fixed kernel-start overhead."""
    main_bb = nc.m.functions[0].blocks[0]
    moved = []
    for f in nc.m.functions:
        for bb in f.blocks:
            if bb is main_bb:
                continue
            insts = bb.instructions
            i = 0
            while i < len(insts):
                if insts[i].name in names:
                    moved.append(insts.pop(i))
                else:
                    i += 1
    pos = 0
    mi = main_bb.instructions
    while pos < len(mi) and mi[pos].opcode in ("Call", "RegisterMove"):
        pos += 1
    for j, ins in enumerate(moved):
        mi.insert(pos + j, ins)
    return len(moved)


def _build():
    f16 = mybir.dt.float16
    f32 = mybir.dt.float32
    i16 = mybir.dt.int16
    op = mybir.AluOpType

    nc = bass.Bass()
    wm = nc.declare_dram_parameter("wm", [NPART, WM_COLS], i16, isOutput=False)
    xsrc = nc.declare_dram_parameter("xsrc", [NB, TOK], i16, isOutput=False)
    out = nc.declare_dram_parameter("out", [TOK, D], f16, isOutput=True)

    hoist_names = []
    with _LeanTailTileContext(nc) as tc:
        with (
            tc.tile_pool(name="const", bufs=1) as cpool,
            tc.tile_pool(name="outp", bufs=8) as opool,
            tc.tile_pool(name="psum", bufs=1, space="PSUM") as ppool,
        ):
            sb = cpool.tile([64, WM_COLS], i16)
            xb = cpool.tile([64, TOK], i16)
            bt = cpool.tile([64, TOK], i16)

            w = sb[:, 0:W_COLS].bitcast(f16)
            shf = sb[:, SHIFT_COL : SHIFT_COL + 1]
            mks = sb[:, MASK_COL : MASK_COL + 1]
            btf = bt.bitcast(f16)

            # input DMAs (hoisted to the preamble by name below), spread
            # over the three DGE queues, first-needed pieces first
            XC = 512  # x columns needed by supertiles 0-1
            dmas = [
                nc.scalar.dma_start(
                    sb[0:NPART, W_COLS:WM_COLS], wm[:, W_COLS:WM_COLS]
                ),
                nc.scalar.dma_start(sb[0:NPART, 0:W_COLS], wm[:, 0:W_COLS]),
                nc.sync.dma_start(xb[0:NB, 0:XC], xsrc[:, 0:XC]),
                nc.sync.dma_start(xb[0:NB, XC:], xsrc[:, XC:]),
                nc.gpsimd.dma_start(xb[32 : 32 + NB, 0:XC], xsrc[:, 0:XC]),
                nc.gpsimd.dma_start(xb[32 : 32 + NB, XC:], xsrc[:, XC:]),
            ]
            hoist_names = [d.ins.name for d in dmas]

            # bits: ONE fused bitwise (x << shift) & mask per chunk on DVE
            # (2x 16-bit mode) writes the fp16 bit pattern directly. Dead
            # rows have mask 0, so their uninitialized x reads land as
            # exact zeros.
            def bits(lo, hi, plo, phi):
                nc.vector.tensor_scalar(
                    bt[plo:phi, lo:hi], xb[plo:phi, lo:hi],
                    shf[plo:phi], mks[plo:phi],
                    op.logical_shift_left, op.bitwise_and,
                )

            def supertile(s):
                ob = opool.tile([TILE, NG * D], f16)
                for g in range(NG):
                    c0 = (s * NG + g) * TILE
                    pt = ppool.tile([TILE, 1024], f32, tag=f"p{g}", bufs=2)
                    for h in range(2):
                        nc.tensor.matmul(
                            pt[:, 512 * h : 512 * (h + 1)],
                            btf[32 * g : 32 * g + NB, c0 : c0 + TILE],
                            w[32 * g : 32 * g + NB, 512 * h : 512 * (h + 1)],
                            start=True,
                            stop=True,
                            tile_position=(32 * g, 0),
                        )
                    dst = ob[:, g * D : (g + 1) * D]
                    if g == 1 and 1 <= s <= NST - 2:
                        nc.vector.tensor_copy(dst, pt[:])
                    else:
                        nc.scalar.copy(dst, pt[:])
                    # per-group DMA right after each copy, on separate
                    # queues so neither group's stream waits on the other's
                    # copy engine (SP for g0; gpsimd SWDGE for g1 except
                    # the final supertile, which stays on SP for the
                    # shortest-latency tail)
                    dv = out[s * ST : (s + 1) * ST, :].rearrange(
                        "(p g) d -> p g d", g=NG
                    )[:, g : g + 1, :]
                    eng = nc.gpsimd if (g == 1 and s < NST - 1) else nc.sync
                    eng.dma_start(dv, dst)

            # interleave bits chunks with the supertile stream so DVE's
            # copy ladder isn't stuck behind bits for late tokens
            bits(0, 512, 0, 32)
            bits(0, 512, 32, 64)
            supertile(0)
            supertile(1)
            bits(512, 1024, 0, 64)
            supertile(2)
            supertile(3)
            bits(1024, 2048, 0, 64)
            supertile(4)
            supertile(5)
            bits(2048, 4096, 0, 64)
            for s in range(6, NST):
                supertile(s)

    _hoist_to_preamble(nc, set(hoist_names))
    _drop_entry_barrier(nc)
    _split_multi_waits(nc)
    return nc


_nc_cache = None


def _make_wm(weight):
    """[NPART, WM_COLS] int16: prescaled fp16 weight rows (bitcast) plus
    per-row left-shift amounts and fp16 single-bit masks, replicated into
    both 32-row groups. Row r's bit lands at fp16 bit position PB[r]
    (pattern 1 << PB[r]); the weight row is prescaled by 1/value(pattern)
    — exact powers of two, no precision loss."""
    wmk = np.zeros((NPART, WM_COLS), np.int16)
    pb = np.array(PB)
    pat_val = np.array(
        [np.frombuffer(np.int16(1 << p).tobytes(), np.float16)[0] for p in pb],
        dtype=np.float32,
    )
    w16 = (np.asarray(weight, dtype=np.float32) / pat_val[:, None]).astype(np.float16)
    shifts = (pb - np.arange(NB)).astype(np.int16)
    masks = (1 << pb).astype(np.int16)
    for g in range(NG):
        wmk[32 * g : 32 * g + NB, 0:W_COLS] = w16.view(np.int16)
        wmk[32 * g : 32 * g + NB, SHIFT_COL] = shifts
        wmk[32 * g : 32 * g + NB, MASK_COL] = masks
    return wmk


def kernel(x, weight):
    global _nc_cache, LAST_RESULTS
    if _nc_cache is None:
        _nc_cache = _build()
    nc = _nc_cache
    wmk = _make_wm(weight)

    # x values are < 8192 so they fit int16 exactly. Within each supertile
    # of 256 tokens, bits column (2s+g)*128 + p must hold token
    # s*256 + 2p + g so each DRAM partition row is 4 KiB contiguous.
    xf = np.asarray(x, dtype=np.int32).reshape(-1).astype(np.int16)
    in_maps = []
    for c in range(N_CORES):
        xs = xf[c * TOK : (c + 1) * TOK]
        xperm = xs.reshape(NST, TILE, NG).swapaxes(1, 2).reshape(-1)
        in_maps.append(
            {
                "wm": wmk,
                "xsrc": np.broadcast_to(xperm, (NB, TOK)).copy(),
            }
        )
    res = run_bass_kernel_spmd(nc, in_maps, list(range(N_CORES)), trace=TRACE)
    LAST_RESULTS = res
    out = np.concatenate([r["out"] for r in res.results], axis=0)
    return out.astype(np.float32).reshape(B, S, D)


# revision 19
# speedup vs baseline: 1.0895x; 1.0332x over previous
"""Binary position embedding kernel for Trainium2, 8-core data-parallel.

out[t, :] = sum_b bit_b(x[t]) * weight[b, :]  ==  bits(x) @ weight

v7: fp16 end-to-end on the device (the 2e-2 rel-err budget dwarfs fp16's
~3e-4), which halves the output stream to 8 MiB/core — the kernel is
DMA-write-bound, so this is the dominant win over the f32 baseline.

Sharding: x flat [32768] -> 8 shards of 4096 tokens; weight replicated.

Per-core plan (4096 tokens -> [4096, 1024] fp16 = 8 MiB output):
  - Two 32-row PE groups (tile rows 0 and 32). Supertile = 256 tokens =
    2 groups x 128; group g computes tokens {s*256 + 2p + g} (host
    permutes x). PSUM: 2 tags x bufs=2 x [128, 1024] f32 (2 banks) =
    all 8 banks, so group g's matmuls for supertile s+1 only wait on
    its copy from supertile s-1 — the mm->copy->mm loop never gates
    the stream.
  - Inputs spread over all 3 DGE queues, hoisted before the Tile entry
    barrier to overlap the fixed ~6.5 us NRT preamble, split so the
    pieces that unblock the first supertile land first: ACT's HWDGE
    queue carries wm [45, 1028] i16 as shift/mask columns (360 B) then
    weight columns; SP's queue carries x cols [0:512] then [512:4096]
    for group 0; gpsimd SWDGE the same two pieces for group 1. Dead
    rows between the groups are never read downstream, so their
    uninitialized x/mask partitions are harmless.
  - bits: ONE fused bitwise DVE tensor_scalar per column chunk,
    (x << shift[p]) & fp16_bit_mask[p], which lands each bit at an fp16
    power-of-two bit position; the host prescales weight rows by the
    matching power of two, so the int16 result bitcast to fp16 IS the
    bit matrix (no arith cast op needed). Chunks are emitted interleaved
    with the copy stream so early copies aren't stuck behind bits for
    late tokens.
  - PSUM -> SBUF copies cast f32 -> fp16, one [128, 1024] copy per
    (s, g), split ACT (g0) / DVE (g1, which also runs bits).
  - Output: token interleave makes partition p's 4 KiB contiguous in
    DRAM, so each supertile is ONE [128, 2048] fp16 DMA (the f32
    baseline's per-row 4 KiB descriptors made the E79 descriptor-
    dispatch engine's per-packet overhead an ~8 us serial tail; 4 KiB
    descriptors with half the packets stay clear of it). The first and
    last supertiles issue per-group DMAs for latency at the stream's
    ends. All output DMAs issue on SP after its single x issue.
"""

import numpy as np

import concourse.bass as bass
import concourse.mybir as mybir
from concourse.bass_utils import run_bass_kernel_spmd
from concourse.tile import TileContext
from concourse.vector_clock import ScopedClock


class _LeanTailTileContext(TileContext):
    """Standard tail emits drain -> barrier -> sem clears -> barrier. The
    final barrier only syncs engine-stream ends after the gpsimd-only sem
    clears; dropping it shaves the second EVSEM butterfly off the critical
    path. Re-execution stays safe: clears still run after the full barrier,
    and the next run's entry barrier resynchronizes engines."""

    def _drain_and_barrier(self, tick_clock, wait_clock):
        nc = self.nc
        drain_inst = nc.sync.drain()
        wait_clock.add_sem_waits(
            drain_inst.ins, ScopedClock({None: tick_clock.global_clock})
        )
        nc.all_engine_barrier()
        popped = nc._tile_sem_poison_stack.pop()
        assert popped is self._sem_poison
        nc.clear_and_free_semaphores(list(self.sems.allocated().values()))


N_CORES = 8
B, S, D = 4, 8192, 1024
NB = 13                    # bits per position
NG = 2                     # PE row groups / token interleave factor
TOK = (B * S) // N_CORES   # 4096 tokens per core
TILE = 128
ST = NG * TILE             # 256 tokens per supertile
NST = TOK // ST            # 16 supertiles
NPART = 32 + NB            # wm rows transferred (45); dead rows 13..31 unread

W_COLS = D                 # wm cols [0:1024] = prescaled fp16 weight bitcast
SHIFT_COL = W_COLS         # wm col 1024 = left-shift amount
MASK_COL = W_COLS + 1      # wm col 1025 = fp16 power-of-two bit pattern
WM_COLS = W_COLS + 4       # pad to 4-col alignment

# Row r carries bit r, left-shifted to fp16 bit position PB so the fused
# bitwise (x << shift) & mask yields the fp16 value of pattern 1<<PB
# directly; the weight row is host-prescaled by 1/that (exact powers of
# two: 2^-14 / 2^-13 / 2^-11 for PB 10 / 11 / 12).
PB = [10] * 11 + [11, 12]

TRACE = False
LAST_RESULTS = None

_wsplit_counter = [0]


def _split_multi_waits(nc):
    """This env's walrus allows only one sync-wait per instruction. Hoist
    extra semaphore waits onto single-wait NoOps inserted just before the
    instruction on the same engine stream (same per-engine program order,
    identical blocking semantics)."""
    import bass_rust

    n_split = 0
    for f in nc.m.functions:
        for bb in f.blocks:
            insts = bb.instructions
            i = 0
            while i < len(insts):
                ins = insts[i]
                si = ins.sync_info
                if si is not None:
                    waits = list(si.on_wait)
                    sem_waits = [w for w in waits if w.sync_type == "semaphore"]
                    other = [w for w in waits if w.sync_type != "semaphore"]
                    keep = 1 if not other else 0
                    if len(waits) > 1 and len(sem_waits) > keep:
                        hoist = sem_waits[: len(sem_waits) - keep]
                        kept = sem_waits[len(sem_waits) - keep:]
                        si.on_wait = other + kept
                        for w in hoist:
                            noop = mybir.InstNoOp(
                                name=f"wsplit-{_wsplit_counter[0]}", ins=[], outs=[]
                            )
                            _wsplit_counter[0] += 1
                            noop.engine = ins.engine
                            noop.sync_info = bass_rust.SyncInfo(
                                on_wait=[w], on_update=[]
                            )
                            insts.insert(i, noop)
                            i += 1
                            n_split += 1
                i += 1
    return n_split


def _drop_entry_barrier(nc):
    """Remove the Tile entry barrier (per-engine Drain + EVSEM butterfly) from
    the preamble block. The preamble's RegisterMoves are same-engine/program-
    order with the body, its memset'd const tiles have no readers, and every
    real cross-engine dependency in the body is semaphore-gated, so the
    barrier only adds latency (~0.2-0.5 us on the critical engine)."""
    main = nc.m.functions[0].blocks[0]
    insts = main.instructions
    i, n = 0, 0
    while i < len(insts):
        ins = insts[i]
        if ins.opcode == "Drain" or ins.name.startswith("barrier_"):
            insts.pop(i)
            n += 1
        else:
            i += 1
    return n


def _hoist_to_preamble(nc, names):
    """Move the named (wait-free) instructions from the body block to the
    preamble block, before the Tile entry barrier, so their DMA transfers
    overlap the fixed kernel-start overhead."""
    main_bb = nc.m.functions[0].blocks[0]
    moved = []
    for f in nc.m.functions:
        for bb in f.blocks:
            if bb is main_bb:
                continue
            insts = bb.instructions
            i = 0
            while i < len(insts):
                if insts[i].name in names:
                    moved.append(insts.pop(i))
                else:
                    i += 1
    pos = 0
    mi = main_bb.instructions
    while pos < len(mi) and mi[pos].opcode in ("Call", "RegisterMove"):
        pos += 1
    for j, ins in enumerate(moved):
        mi.insert(pos + j, ins)
    return len(moved)


def _build():
    f16 = mybir.dt.float16
    f32 = mybir.dt.float32
    i16 = mybir.dt.int16
    op = mybir.AluOpType

    nc = bass.Bass()
    wm = nc.declare_dram_parameter("wm", [NPART, WM_COLS], i16, isOutput=False)
    xsrc = nc.declare_dram_parameter("xsrc", [NB, TOK], i16, isOutput=False)
    out = nc.declare_dram_parameter("out", [TOK, D], f16, isOutput=True)

    hoist_names = []
    with _LeanTailTileContext(nc) as tc:
        with (
            tc.tile_pool(name="const", bufs=1) as cpool,
            tc.tile_pool(name="outp", bufs=8) as opool,
            tc.tile_pool(name="psum", bufs=1, space="PSUM") as ppool,
        ):
            sb = cpool.tile([64, WM_COLS], i16)
            xb = cpool.tile([64, TOK], i16)
            bt = cpool.tile([64, TOK], i16)

            w = sb[:, 0:W_COLS].bitcast(f16)
            shf = sb[:, SHIFT_COL : SHIFT_COL + 1]
            mks = sb[:, MASK_COL : MASK_COL + 1]
            btf = bt.bitcast(f16)

            # input DMAs (hoisted to the preamble by name below), spread
            # over the three DGE queues, first-needed pieces first: the
            # tiny shift/mask columns and the first supertiles' x columns
            # unblock bits/matmuls while the bulk still streams
            XC = 512  # x columns needed by supertiles 0-1
            dmas = [
                nc.scalar.dma_start(
                    sb[0:NPART, W_COLS:WM_COLS], wm[:, W_COLS:WM_COLS]
                ),
                nc.scalar.dma_start(sb[0:NPART, 0:W_COLS], wm[:, 0:W_COLS]),
                nc.sync.dma_start(xb[0:NB, 0:XC], xsrc[:, 0:XC]),
                nc.sync.dma_start(xb[0:NB, XC:], xsrc[:, XC:]),
                nc.gpsimd.dma_start(xb[32 : 32 + NB, 0:XC], xsrc[:, 0:XC]),
                nc.gpsimd.dma_start(xb[32 : 32 + NB, XC:], xsrc[:, XC:]),
            ]
            hoist_names = [d.ins.name for d in dmas]

            # bits: ONE fused bitwise (x << shift) & mask per chunk on DVE
            # (2x 16-bit mode) writes the fp16 bit pattern directly. Dead
            # rows have mask 0, so their uninitialized x reads land as
            # exact zeros.
            def bits(lo, hi, plo, phi):
                nc.vector.tensor_scalar(
                    bt[plo:phi, lo:hi], xb[plo:phi, lo:hi],
                    shf[plo:phi], mks[plo:phi],
                    op.logical_shift_left, op.bitwise_and,
                )

            def supertile(s):
                ob = opool.tile([TILE, NG * D], f16)
                per_group_dma = s in (0, 1, NST - 1)
                for g in range(NG):
                    c0 = (s * NG + g) * TILE
                    pt = ppool.tile([TILE, 1024], f32, tag=f"p{g}", bufs=2)
                    for h in range(2):
                        nc.tensor.matmul(
                            pt[:, 512 * h : 512 * (h + 1)],
                            btf[32 * g : 32 * g + NB, c0 : c0 + TILE],
                            w[32 * g : 32 * g + NB, 512 * h : 512 * (h + 1)],
                            start=True,
                            stop=True,
                            tile_position=(32 * g, 0),
                        )
                    dst = ob[:, g * D : (g + 1) * D]
                    if g == 1 and 1 <= s <= NST - 2:
                        nc.vector.tensor_copy(dst, pt[:])
                    else:
                        nc.scalar.copy(dst, pt[:])
                    if per_group_dma:
                        dv = out[s * ST : (s + 1) * ST, :].rearrange(
                            "(p g) d -> p g d", g=NG
                        )[:, g : g + 1, :]
                        nc.sync.dma_start(dv, dst)
                if not per_group_dma:
                    dram_view = out[s * ST : (s + 1) * ST, :].rearrange(
                        "(p g) d -> p (g d)", g=NG
                    )
                    nc.sync.dma_start(dram_view, ob[:])

            # interleave bits chunks with the supertile stream so DVE's
            # copy ladder isn't stuck behind bits for late tokens
            bits(0, 512, 0, 32)
            bits(0, 512, 32, 64)
            supertile(0)
            supertile(1)
            bits(512, 1024, 0, 64)
            supertile(2)
            supertile(3)
            bits(1024, 2048, 0, 64)
            supertile(4)
            supertile(5)
            bits(2048, 4096, 0, 64)
            for s in range(6, NST):
                supertile(s)

    _hoist_to_preamble(nc, set(hoist_names))
    _drop_entry_barrier(nc)
    _split_multi_waits(nc)
    return nc


_nc_cache = None


def _make_wm(weight):
    """[NPART, WM_COLS] int16: prescaled fp16 weight rows (bitcast) plus
    per-row left-shift amounts and fp16 single-bit masks, replicated into
    both 32-row groups. Row r's bit lands at fp16 bit position PB[r]
    (pattern 1 << PB[r]); the weight row is prescaled by 1/value(pattern)
    — exact powers of two, no precision loss."""
    wmk = np.zeros((NPART, WM_COLS), np.int16)
    pb = np.array(PB)
    pat_val = np.array(
        [np.frombuffer(np.int16(1 << p).tobytes(), np.float16)[0] for p in pb],
        dtype=np.float32,
    )
    w16 = (np.asarray(weight, dtype=np.float32) / pat_val[:, None]).astype(np.float16)
    shifts = (pb - np.arange(NB)).astype(np.int16)
    masks = (1 << pb).astype(np.int16)
    for g in range(NG):
        wmk[32 * g : 32 * g + NB, 0:W_COLS] = w16.view(np.int16)
        wmk[32 * g : 32 * g + NB, SHIFT_COL] = shifts
        wmk[32 * g : 32 * g + NB, MASK_COL] = masks
    return wmk


def kernel(x, weight):
    global _nc_cache, LAST_RESULTS
    if _nc_cache is None:
        _nc_cache = _build()
    nc = _nc_cache
    wmk = _make_wm(weight)

    # x values are < 8192 so they fit int16 exactly. Within each supertile
    # of 256 tokens, bits column (2s+g)*128 + p must hold token
    # s*256 + 2p + g so each DRAM partition row is 4 KiB contiguous.
    xf = np.asarray(x, dtype=np.int32).reshape(-1).astype(np.int16)
    in_maps = []
    for c in range(N_CORES):
        xs = xf[c * TOK : (c + 1) * TOK]
        xperm = xs.reshape(NST, TILE, NG).swapaxes(1, 2).reshape(-1)
        in_maps.append(
            {
                "wm": wmk,
                "xsrc": np.broadcast_to(xperm, (NB, TOK)).copy(),
            }
        )
    res = run_bass_kernel_spmd(nc, in_maps, list(range(N_CORES)), trace=TRACE)
    LAST_RESULTS = res
    out = np.concatenate([r["out"] for r in res.results], axis=0)
    return out.astype(np.float32).reshape(B, S, D)


# revision 20
# speedup vs baseline: 1.2120x; 1.1124x over previous
"""Binary position embedding kernel for Trainium2, 8-core data-parallel.

out[t, :] = sum_b bit_b(x[t]) * weight[b, :]  ==  bits(x) @ weight

v4: fp16 end-to-end on the device (the 2e-2 rel-err budget dwarfs fp16's
~3e-4), which halves the output stream to 8 MiB/core — the kernel is
DMA-write-bound, so this is the dominant win over the f32 baseline.

Sharding: x flat [32768] -> 8 shards of 4096 tokens; weight replicated.

Per-core plan (4096 tokens -> [4096, 1024] fp16 = 8 MiB output):
  - Two 32-row PE groups (tile rows 0 and 32). Supertile = 256 tokens =
    2 groups x 128; group g computes tokens {s*256 + 2p + g} (host
    permutes x). PSUM: 2 tags x bufs=2 x [128, 1024] f32 (2 banks) =
    all 8 banks, so group g's matmuls for supertile s+1 only wait on
    its copy from supertile s-1 — the mm->copy->mm loop never gates
    the stream.
  - Inputs: just 3 DMAs, one per DGE queue, all hoisted before the Tile
    entry barrier to overlap the fixed ~6.5 us NRT preamble: wm [64,
    1028] i16 (prescaled fp16 weight rows bitcast + per-row shift/mask,
    both groups) on ACT's HWDGE queue; x [13, 4096] i16 to group 0's
    partitions on SP's queue and group 1's via gpsimd SWDGE. Dead rows
    have mask 0, which zeroes whatever garbage their uninitialized x
    partitions hold.
  - bits: ONE fused bitwise DVE tensor_scalar per column chunk,
    (x << shift[p]) & fp16_bit_mask[p], which lands each bit at an fp16
    power-of-two bit position; the host prescales weight rows by the
    matching power of two, so the int16 result bitcast to fp16 IS the
    bit matrix (no arith cast op needed). Chunks are emitted interleaved
    with the copy stream so early copies aren't stuck behind bits for
    late tokens.
  - PSUM -> SBUF copies cast f32 -> fp16, one [128, 1024] copy per
    (s, g), split ACT (g0) / DVE (g1, which also runs bits).
  - Output: token interleave makes partition p's 4 KiB contiguous in
    DRAM, so each supertile is ONE [128, 2048] fp16 DMA (the f32
    baseline's per-row 4 KiB descriptors made the E79 descriptor-
    dispatch engine's per-packet overhead an ~8 us serial tail; 4 KiB
    descriptors with half the packets stay clear of it). The first and
    last supertiles issue per-group DMAs for latency at the stream's
    ends. All output DMAs issue on SP after its single x issue.
"""

import numpy as np

import concourse.bass as bass
import concourse.mybir as mybir
from concourse.bass_utils import run_bass_kernel_spmd
from concourse.tile import TileContext
from concourse.vector_clock import ScopedClock


class _LeanTailTileContext(TileContext):
    """Standard tail emits drain -> barrier -> sem clears -> barrier. The
    final barrier only syncs engine-stream ends after the gpsimd-only sem
    clears; dropping it shaves the second EVSEM butterfly off the critical
    path. Re-execution stays safe: clears still run after the full barrier,
    and the next run's entry barrier resynchronizes engines."""

    def _drain_and_barrier(self, tick_clock, wait_clock):
        nc = self.nc
        drain_inst = nc.sync.drain()
        wait_clock.add_sem_waits(
            drain_inst.ins, ScopedClock({None: tick_clock.global_clock})
        )
        nc.all_engine_barrier()
        popped = nc._tile_sem_poison_stack.pop()
        assert popped is self._sem_poison
        nc.clear_and_free_semaphores(list(self.sems.allocated().values()))


N_CORES = 8
B, S, D = 4, 8192, 1024
NB = 13                    # bits per position
NG = 2                     # PE row groups / token interleave factor
TOK = (B * S) // N_CORES   # 4096 tokens per core
TILE = 128
ST = NG * TILE             # 256 tokens per supertile
NST = TOK // ST            # 16 supertiles
NPART = 64                 # both groups (dead rows zero-filled)

W_COLS = D                 # wm cols [0:1024] = prescaled fp16 weight bitcast
SHIFT_COL = W_COLS         # wm col 1024 = left-shift amount
MASK_COL = W_COLS + 1      # wm col 1025 = fp16 power-of-two bit pattern
WM_COLS = W_COLS + 4       # pad to 4-col alignment

# Row r carries bit r, left-shifted to fp16 bit position PB so the fused
# bitwise (x << shift) & mask yields the fp16 value of pattern 1<<PB
# directly; the weight row is host-prescaled by 1/that (exact powers of
# two: 2^-14 / 2^-13 / 2^-11 for PB 10 / 11 / 12).
PB = [10] * 11 + [11, 12]

TRACE = False
LAST_RESULTS = None

_wsplit_counter = [0]


def _split_multi_waits(nc):
    """This env's walrus allows only one sync-wait per instruction. Hoist
    extra semaphore waits onto single-wait NoOps inserted just before the
    instruction on the same engine stream (same per-engine program order,
    identical blocking semantics)."""
    import bass_rust

    n_split = 0
    for f in nc.m.functions:
        for bb in f.blocks:
            insts = bb.instructions
            i = 0
            while i < len(insts):
                ins = insts[i]
                si = ins.sync_info
                if si is not None:
                    waits = list(si.on_wait)
                    sem_waits = [w for w in waits if w.sync_type == "semaphore"]
                    other = [w for w in waits if w.sync_type != "semaphore"]
                    keep = 1 if not other else 0
                    if len(waits) > 1 and len(sem_waits) > keep:
                        hoist = sem_waits[: len(sem_waits) - keep]
                        kept = sem_waits[len(sem_waits) - keep:]
                        si.on_wait = other + kept
                        for w in hoist:
                            noop = mybir.InstNoOp(
                                name=f"wsplit-{_wsplit_counter[0]}", ins=[], outs=[]
                            )
                            _wsplit_counter[0] += 1
                            noop.engine = ins.engine
                            noop.sync_info = bass_rust.SyncInfo(
                                on_wait=[w], on_update=[]
                            )
                            insts.insert(i, noop)
                            i += 1
                            n_split += 1
                i += 1
    return n_split


def _drop_entry_barrier(nc):
    """Remove the Tile entry barrier (per-engine Drain + EVSEM butterfly) from
    the preamble block. The preamble's RegisterMoves are same-engine/program-
    order with the body, its memset'd const tiles have no readers, and every
    real cross-engine dependency in the body is semaphore-gated, so the
    barrier only adds latency (~0.2-0.5 us on the critical engine)."""
    main = nc.m.functions[0].blocks[0]
    insts = main.instructions
    i, n = 0, 0
    while i < len(insts):
        ins = insts[i]
        if ins.opcode == "Drain" or ins.name.startswith("barrier_"):
            insts.pop(i)
            n += 1
        else:
            i += 1
    return n


def _hoist_to_preamble(nc, names):
    """Move the named (wait-free) instructions from the body block to the
    preamble block, before the Tile entry barrier, so their DMA transfers
    overlap the fixed kernel-start overhead."""
    main_bb = nc.m.functions[0].blocks[0]
    moved = []
    for f in nc.m.functions:
        for bb in f.blocks:
            if bb is main_bb:
                continue
            insts = bb.instructions
            i = 0
            while i < len(insts):
                if insts[i].name in names:
                    moved.append(insts.pop(i))
                else:
                    i += 1
    pos = 0
    mi = main_bb.instructions
    while pos < len(mi) and mi[pos].opcode in ("Call", "RegisterMove"):
        pos += 1
    for j, ins in enumerate(moved):
        mi.insert(pos + j, ins)
    return len(moved)


def _build():
    f16 = mybir.dt.float16
    f32 = mybir.dt.float32
    i16 = mybir.dt.int16
    op = mybir.AluOpType

    nc = bass.Bass()
    wm = nc.declare_dram_parameter("wm", [NPART, WM_COLS], i16, isOutput=False)
    xsrc = nc.declare_dram_parameter("xsrc", [NB, TOK], i16, isOutput=False)
    out = nc.declare_dram_parameter("out", [TOK, D], f16, isOutput=True)

    hoist_names = []
    with _LeanTailTileContext(nc) as tc:
        with (
            tc.tile_pool(name="const", bufs=1) as cpool,
            tc.tile_pool(name="outp", bufs=8) as opool,
            tc.tile_pool(name="psum", bufs=1, space="PSUM") as ppool,
        ):
            sb = cpool.tile([NPART, WM_COLS], i16)
            xb = cpool.tile([64, TOK], i16)
            bt = cpool.tile([64, TOK], i16)

            w = sb[:, 0:W_COLS].bitcast(f16)
            shf = sb[:, SHIFT_COL : SHIFT_COL + 1]
            mks = sb[:, MASK_COL : MASK_COL + 1]
            btf = bt.bitcast(f16)

            # input DMAs (hoisted to the preamble by name below): one per
            # DGE queue so nothing serializes
            d0 = nc.scalar.dma_start(sb[:], wm[:])
            d1 = nc.sync.dma_start(xb[0:NB, :], xsrc[:])
            d2 = nc.gpsimd.dma_start(xb[32 : 32 + NB, :], xsrc[:])
            hoist_names = [d0.ins.name, d1.ins.name, d2.ins.name]

            # bits: ONE fused bitwise (x << shift) & mask per chunk on DVE
            # (2x 16-bit mode) writes the fp16 bit pattern directly. Dead
            # rows have mask 0, so their uninitialized x reads land as
            # exact zeros.
            def bits(lo, hi, plo, phi):
                nc.vector.tensor_scalar(
                    bt[plo:phi, lo:hi], xb[plo:phi, lo:hi],
                    shf[plo:phi], mks[plo:phi],
                    op.logical_shift_left, op.bitwise_and,
                )

            def supertile(s):
                ob = opool.tile([TILE, NG * D], f16)
                per_group_dma = s in (0, 1, NST - 1)
                for g in range(NG):
                    c0 = (s * NG + g) * TILE
                    pt = ppool.tile([TILE, 1024], f32, tag=f"p{g}", bufs=2)
                    for h in range(2):
                        nc.tensor.matmul(
                            pt[:, 512 * h : 512 * (h + 1)],
                            btf[32 * g : 32 * g + NB, c0 : c0 + TILE],
                            w[32 * g : 32 * g + NB, 512 * h : 512 * (h + 1)],
                            start=True,
                            stop=True,
                            tile_position=(32 * g, 0),
                        )
                    dst = ob[:, g * D : (g + 1) * D]
                    if g == 1 and 1 <= s <= NST - 2:
                        nc.vector.tensor_copy(dst, pt[:])
                    else:
                        nc.scalar.copy(dst, pt[:])
                    if per_group_dma:
                        dv = out[s * ST : (s + 1) * ST, :].rearrange(
                            "(p g) d -> p g d", g=NG
                        )[:, g : g + 1, :]
                        nc.sync.dma_start(dv, dst)
                if not per_group_dma:
                    dram_view = out[s * ST : (s + 1) * ST, :].rearrange(
                        "(p g) d -> p (g d)", g=NG
                    )
                    nc.sync.dma_start(dram_view, ob[:])

            # interleave bits chunks with the supertile stream so DVE's
            # copy ladder isn't stuck behind bits for late tokens
            bits(0, 512, 0, 32)
            bits(0, 512, 32, 64)
            supertile(0)
            supertile(1)
            bits(512, 1024, 0, 64)
            supertile(2)
            supertile(3)
            bits(1024, 2048, 0, 64)
            supertile(4)
            supertile(5)
            bits(2048, 4096, 0, 64)
            for s in range(6, NST):
                supertile(s)

    _hoist_to_preamble(nc, set(hoist_names))
    _drop_entry_barrier(nc)
    _split_multi_waits(nc)
    return nc


_nc_cache = None


def _make_wm(weight):
    """[NPART, WM_COLS] int16: prescaled fp16 weight rows (bitcast) plus
    per-row left-shift amounts and fp16 single-bit masks, replicated into
    both 32-row groups. Row r's bit lands at fp16 bit position PB[r]
    (pattern 1 << PB[r]); the weight row is prescaled by 1/value(pattern)
    — exact powers of two, no precision loss."""
    wmk = np.zeros((NPART, WM_COLS), np.int16)
    pb = np.array(PB)
    pat_val = np.array(
        [np.frombuffer(np.int16(1 << p).tobytes(), np.float16)[0] for p in pb],
        dtype=np.float32,
    )
    w16 = (np.asarray(weight, dtype=np.float32) / pat_val[:, None]).astype(np.float16)
    shifts = (pb - np.arange(NB)).astype(np.int16)
    masks = (1 << pb).astype(np.int16)
    for g in range(NG):
        wmk[32 * g : 32 * g + NB, 0:W_COLS] = w16.view(np.int16)
        wmk[32 * g : 32 * g + NB, SHIFT_COL] = shifts
        wmk[32 * g : 32 * g + NB, MASK_COL] = masks
    return wmk


def kernel(x, weight):
    global _nc_cache, LAST_RESULTS
    if _nc_cache is None:
        _nc_cache = _build()
    nc = _nc_cache
    wmk = _make_wm(weight)

    # x values are < 8192 so they fit int16 exactly. Within each supertile
    # of 256 tokens, bits column (2s+g)*128 + p must hold token
    # s*256 + 2p + g so each DRAM partition row is 4 KiB contiguous.
    xf = np.asarray(x, dtype=np.int32).reshape(-1).astype(np.int16)
    in_maps = []
    for c in range(N_CORES):
        xs = xf[c * TOK : (c + 1) * TOK]
        xperm = xs.reshape(NST, TILE, NG).swapaxes(1, 2).reshape(-1)
        in_maps.append(
            {
                "wm": wmk,
                "xsrc": np.broadcast_to(xperm, (NB, TOK)).copy(),
            }
        )
    res = run_bass_kernel_spmd(nc, in_maps, list(range(N_CORES)), trace=TRACE)
    LAST_RESULTS = res
    out = np.concatenate([r["out"] for r in res.results], axis=0)
    return out.astype(np.float32).reshape(B, S, D)
